# revision 11
# baseline (speedup 1.0000x reference)
"""Trainium2 Bass kernel for a BasicTransformerBlock (B=2, S=2048, H=768, FF=3072, NH=12).

Sharding: core c handles batch b=c//4, sequence quarter q=c%4 (512 tokens).
Each core redundantly computes LN1 + K/V projections for its batch's full
2048 tokens (no collectives needed); Q/attention/Wo/FFN only for its own 512
tokens.  Activations are kept feature-major ([feature, token]) on chip;
attention scores are computed transposed ([tk, tq]) so softmax reduces over
the partition dim via M=1 ones-matmuls (rowsums) and a deferred normalize.

LN affine params and all biases are folded host-side:
  Wq_eff = diag(ln1_w) Wq, bq_eff = ln1_b@Wq + bq  (same k)
  v carries no bias on device;  bo_eff = (ln1_b@Wv + bv)@Wo + bo
  W1_eff = diag(ln2_w) W1, b1_eff = ln2_b@W1 + b1
"""

import numpy as np
import ml_dtypes

import concourse.bass as bass
import concourse.tile as tile
from concourse import bacc, mybir
from concourse.bass import ts, ds
from concourse.alu_op_type import AluOpType
from concourse.bass_utils import run_bass_kernel_spmd

F32 = mybir.dt.float32
F32R = mybir.dt.float32r
BF16 = mybir.dt.bfloat16
AF = mybir.ActivationFunctionType

H = 768
FF = 3072
NH = 12
DH = 64
B = 2
S = 2048
P = 128
NCORES = 8
TQ = 512          # own tokens per core
NTT = S // TQ     # 4 token tiles per batch
FC = H // P       # 6 feature chunks
FFC = FF // P     # 24 hidden chunks
TKC = S // P      # 16 key token chunks
HPAIRS = NH // 2  # 6 head pairs
EPS = 1e-6


def _emit_ln(nc, T, lat_ap, nx_tile, lat_bf_tile, sq_tile, tmp_pool, small_pool,
             ab_pool, ps_stats_pool, ones_col_bf, eps_tile):
    """Feature-major layernorm: lat_ap/[P,FC,T] f32 -> nx_tile [P,FC,T] bf16."""
    # bf16 copies for the PE stat reductions (ACT + DVE)
    nc.scalar.copy(lat_bf_tile[:], lat_ap)
    nc.vector.tensor_mul(sq_tile[:], lat_ap, lat_ap)
    # per-token sum and sqsum via ones-matmuls accumulated in PSUM
    ps_stat = ps_stats_pool.tile([33, T], F32, tag="stats")
    for c in range(FC):
        nc.tensor.matmul(ps_stat[0:1, :], ones_col_bf[:],
                         lat_bf_tile[:, c, :],
                         start=(c == 0), stop=(c == FC - 1))
    for c in range(FC):
        nc.tensor.matmul(ps_stat[32:33, :], ones_col_bf[:],
                         sq_tile[:, c, :],
                         start=(c == 0), stop=(c == FC - 1))
    # mu, var, rsig, beta  (tiny [1,T] ops)
    mu = small_pool.tile([1, T], F32, tag="mu")
    nc.vector.tensor_scalar_mul(mu[:], ps_stat[0:1, :], 1.0 / H)
    msq = small_pool.tile([1, T], F32, tag="msq")
    nc.vector.tensor_scalar_mul(msq[:], ps_stat[32:33, :], 1.0 / H)
    var = small_pool.tile([1, T], F32, tag="var")
    nc.vector.tensor_mul(var[:], mu[:], mu[:])
    nc.vector.tensor_sub(var[:], msq[:], var[:])
    sd = small_pool.tile([1, T], F32, tag="sd")
    nc.scalar.activation(sd[:], var[:], AF.Sqrt, bias=eps_tile[:])
    rsig = small_pool.tile([1, T], F32, tag="rsig")
    nc.vector.reciprocal(rsig[:], sd[:])
    beta = small_pool.tile([1, T], F32, tag="beta")
    nc.vector.scalar_tensor_tensor(beta[:], mu[:], -1.0, rsig[:],
                                   AluOpType.mult, AluOpType.mult)
    # broadcast alpha/beta along partitions (GPSIMD)
    ab = ab_pool.tile([P, T], F32, tag="ab")
    nc.gpsimd.partition_broadcast(ab[:], rsig[:])
    bb = ab_pool.tile([P, T], F32, tag="bb")
    nc.gpsimd.partition_broadcast(bb[:], beta[:])
    # apply: nx = lat * ab + bb   (per chunk, DVE)
    for c in range(FC):
        t = tmp_pool.tile([P, T], F32, tag="lntmp")
        nc.vector.tensor_mul(t[:], lat_ap[:, c, :], ab[:])
        nc.vector.tensor_add(nx_tile[:, c, :], t[:], bb[:])


def build():
    nc = bacc.Bacc("TRN2", target_bir_lowering=False, debug=False,
                   num_devices=NCORES)

    latT_d = nc.dram_tensor("latT", [H, S], F32, kind="ExternalInput")
    wq_d = nc.dram_tensor("wq", [H, H], BF16, kind="ExternalInput")
    wk_d = nc.dram_tensor("wk", [H, H], BF16, kind="ExternalInput")
    wv_d = nc.dram_tensor("wv", [H, H], BF16, kind="ExternalInput")
    wo_d = nc.dram_tensor("wo", [H, H], BF16, kind="ExternalInput")
    w1_d = nc.dram_tensor("w1r", [FFC, H, P], BF16, kind="ExternalInput")
    w2_d = nc.dram_tensor("w2", [FF, H], BF16, kind="ExternalInput")
    bq_d = nc.dram_tensor("bq", [P, FC], F32, kind="ExternalInput")
    bk_d = nc.dram_tensor("bk", [P, FC], F32, kind="ExternalInput")
    bo_d = nc.dram_tensor("bo", [P, FC], F32, kind="ExternalInput")
    b1_d = nc.dram_tensor("b1", [P, FFC], F32, kind="ExternalInput")
    b2_d = nc.dram_tensor("b2", [P, FC], F32, kind="ExternalInput")
    out_d = nc.dram_tensor("outT", [H, TQ], F32, kind="ExternalOutput")

    latT_ap = latT_d.ap().rearrange("(c p) t -> p c t", p=P)
    out_ap = out_d.ap().rearrange("(c p) t -> p c t", p=P)

    with tile.TileContext(nc) as tc:
        with (
            tc.tile_pool(name="consts", bufs=1) as consts,
            tc.tile_pool(name="persist", bufs=1) as persist,
        ):
            # constants
            ones_col_bf = consts.tile([P, 1], BF16)
            nc.vector.memset(ones_col_bf[:], 1.0)
            eps_tile = consts.tile([1, 1], F32)
            nc.vector.memset(eps_tile[:], EPS)
            zero_col = consts.tile([P, 1], F32)
            nc.vector.memset(zero_col[:], 0.0)
            bq_sb = consts.tile([P, FC], F32)
            nc.sync.dma_start(bq_sb[:], bq_d.ap())
            bk_sb = consts.tile([P, FC], F32)
            nc.sync.dma_start(bk_sb[:], bk_d.ap())
            bo_sb = consts.tile([P, FC], F32)
            nc.sync.dma_start(bo_sb[:], bo_d.ap())
            b1_sb = consts.tile([P, FFC], F32)
            nc.sync.dma_start(b1_sb[:], b1_d.ap())
            b2_sb = consts.tile([P, FC], F32)
            nc.sync.dma_start(b2_sb[:], b2_d.ap())

            # persistent activations
            kT = persist.tile([P, FC, S], BF16)       # K^T, full batch
            v_sb = persist.tile([P, TKC, NH, DH], BF16)  # V token-major
            qT = persist.tile([P, FC, TQ], BF16)      # Q^T, own tokens
            ctxT = persist.tile([P, FC, TQ], BF16)    # attention output^T
            resid1 = persist.tile([P, FC, TQ], F32)   # latT own tokens (f32)

            # projection weights
            wq_sb = persist.tile([P, FC, H], BF16)
            nc.sync.dma_start(wq_sb[:], wq_d.ap().rearrange("(c p) m -> p c m", p=P))
            wk_sb = persist.tile([P, FC, H], BF16)
            nc.sync.dma_start(wk_sb[:], wk_d.ap().rearrange("(c p) m -> p c m", p=P))
            wv_sb = persist.tile([P, FC, H], BF16)
            nc.sync.dma_start(wv_sb[:], wv_d.ap().rearrange("(c p) m -> p c m", p=P))
            wo_sb = persist.tile([P, FC, H], BF16)
            nc.sync.dma_start(wo_sb[:], wo_d.ap().rearrange("(c p) m -> p c m", p=P))

            # ---------------- Phase 1: LN1 + K/V/Q projections ----------------
            with (
                tc.tile_pool(name="latp", bufs=2) as latp,
                tc.tile_pool(name="sqp", bufs=1) as sqp,
                tc.tile_pool(name="nxp", bufs=2) as nxp,
                tc.tile_pool(name="abp", bufs=2) as abp,
                tc.tile_pool(name="smallp", bufs=2) as smallp,
                tc.tile_pool(name="lntmpp", bufs=2) as lntmpp,
                tc.tile_pool(name="ps_stats", bufs=2, space="PSUM") as ps_stats,
                tc.tile_pool(name="ps_kq", bufs=2, space="PSUM") as ps_kq,
                tc.tile_pool(name="ps_v", bufs=2, space="PSUM") as ps_v,
            ):
                for tt in range(NTT):
                    if tt == 0:
                        lat_t = resid1
                        nc.sync.dma_start(lat_t[:], latT_ap[:, :, ts(tt, TQ)])
                    else:
                        lat_t = latp.tile([P, FC, TQ], F32, tag="lat")
                        nc.sync.dma_start(lat_t[:], latT_ap[:, :, ts(tt, TQ)])
                    sq_t = sqp.tile([P, FC, TQ], BF16, tag="sq")
                    latbf_t = sqp.tile([P, FC, TQ], BF16, tag="latbf")
                    nx_t = nxp.tile([P, FC, TQ], BF16, tag="nx")
                    _emit_ln(nc, TQ, lat_t[:], nx_t, latbf_t, sq_t, lntmpp,
                             smallp, abp, ps_stats, ones_col_bf, eps_tile)

                    # K projection (feature-major out)
                    for mc in range(FC):
                        ps = ps_kq.tile([P, TQ], F32, tag="kq")
                        for kc in range(FC):
                            nc.tensor.matmul(ps[:], wk_sb[:, kc, ts(mc, P)],
                                             nx_t[:, kc, :],
                                             start=(kc == 0), stop=(kc == FC - 1))
                        nc.scalar.activation(kT[:, mc, ts(tt, TQ)], ps[:],
                                             AF.Identity, bias=bk_sb[:, mc:mc + 1])
                    # V projection (token-major out)
                    for tcl in range(TQ // P):
                        tcg = tt * (TQ // P) + tcl
                        for half in range(2):
                            ps = ps_v.tile([P, 384], F32, tag="v")
                            for kc in range(FC):
                                nc.tensor.matmul(ps[:], nx_t[:, kc, ts(tcl, P)],
                                                 wv_sb[:, kc, ds(half * 384, 384)],
                                                 start=(kc == 0), stop=(kc == FC - 1))
                            nc.vector.tensor_copy(
                                v_sb[:, tcg, ds(half * 6, 6), :],
                                ps[:].rearrange("p (h d) -> p h d", d=DH))
                    # Q projection (own tokens only; tt==0 holds own tokens)
                    if tt == 0:
                        for mc in range(FC):
                            ps = ps_kq.tile([P, TQ], F32, tag="kq")
                            for kc in range(FC):
                                nc.tensor.matmul(ps[:], wq_sb[:, kc, ts(mc, P)],
                                                 nx_t[:, kc, :],
                                                 start=(kc == 0), stop=(kc == FC - 1))
                            nc.scalar.activation(qT[:, mc, :], ps[:],
                                                 AF.Identity, bias=bq_sb[:, mc:mc + 1])

            # ---------------- Phase 2: attention ----------------
            with (
                tc.tile_pool(name="attnp", bufs=8) as attnp,
                tc.tile_pool(name="rssb", bufs=2) as rssb,
                tc.tile_pool(name="rbp", bufs=2) as rbp,
                tc.tile_pool(name="ps_sc", bufs=4, space="PSUM") as ps_sc,
                tc.tile_pool(name="ps_ctx", bufs=1, space="PSUM") as ps_ctx,
                tc.tile_pool(name="ps_rs", bufs=1, space="PSUM") as ps_rs,
            ):
                for hp in range(HPAIRS):
                    hA, hB = 2 * hp, 2 * hp + 1
                    # separate banks per accumulation group (bank-wide
                    # has_written clear forbids sharing); partition offsets
                    # chosen so every downstream op is lane-aligned.
                    ctxA_ps = ps_ctx.tile([P, TQ], F32, tag="ctxA")
                    ctxB_ps = ps_ctx.tile([P, TQ], F32, tag="ctxB")
                    rsA_ps = ps_rs.tile([1, TQ], F32, tag="rsA")
                    rsB_ps = ps_rs.tile([33, TQ], F32, tag="rsB")
                    for j in range(TKC):
                        psA = ps_sc.tile([P, TQ], F32, tag="sc")
                        nc.tensor.matmul(psA[:], kT[0:DH, hp, ts(j, P)],
                                         qT[0:DH, hp, :], start=True, stop=True)
                        psB = ps_sc.tile([P, TQ], F32, tag="sc")
                        nc.tensor.matmul(psB[:], kT[DH:P, hp, ts(j, P)],
                                         qT[DH:P, hp, :], start=True, stop=True)
                        aA = attnp.tile([P, TQ], BF16, tag="attn")
                        nc.scalar.activation(aA[:], psA[:], AF.Exp, scale=0.125,
                                             bias=zero_col[:])
                        aB = attnp.tile([P, TQ], BF16, tag="attn")
                        nc.scalar.activation(aB[:], psB[:], AF.Exp, scale=0.125,
                                             bias=zero_col[:])
                        # ctx accumulation (head A -> partitions 0:64, B -> 64:128)
                        nc.tensor.matmul(ctxA_ps[0:DH, :], v_sb[:, j, hA, :], aA[:],
                                         start=(j == 0), stop=(j == TKC - 1))
                        nc.tensor.matmul(ctxB_ps[DH:P, :], v_sb[:, j, hB, :], aB[:],
                                         start=(j == 0), stop=(j == TKC - 1))
                        # rowsums
                        nc.tensor.matmul(rsA_ps[0:1, :], ones_col_bf[:], aA[:],
                                         start=(j == 0), stop=(j == TKC - 1))
                        nc.tensor.matmul(rsB_ps[32:33, :], ones_col_bf[:], aB[:],
                                         start=(j == 0), stop=(j == TKC - 1))
                    # normalize:  ctxT[:,hp,:] = ctx * (1/rowsum) broadcast
                    # (HW partition_broadcast only supports partition-0 input
                    # and full base-0 output, so B's row hops to p0 via DMA.)
                    rsA_t = rssb.tile([1, TQ], F32, tag="rsA_sb")
                    nc.vector.reciprocal(rsA_t[0:1, :], rsA_ps[0:1, :])
                    rsB_t = rssb.tile([33, TQ], F32, tag="rsB_sb")
                    nc.vector.reciprocal(rsB_t[32:33, :], rsB_ps[32:33, :])
                    rsB0_t = rssb.tile([1, TQ], F32, tag="rsB0_sb")
                    nc.sync.dma_start(rsB0_t[0:1, :], rsB_t[32:33, :])
                    rbA = rbp.tile([P, TQ], F32, tag="rbA")
                    nc.gpsimd.partition_broadcast(rbA[:], rsA_t[0:1, :])
                    rbB = rbp.tile([P, TQ], F32, tag="rbB")
                    nc.gpsimd.partition_broadcast(rbB[:], rsB0_t[0:1, :])
                    nc.vector.tensor_mul(ctxT[0:DH, hp, :], ctxA_ps[0:DH, :],
                                         rbA[0:DH, :])
                    nc.vector.tensor_mul(ctxT[DH:P, hp, :], ctxB_ps[DH:P, :],
                                         rbB[DH:P, :])

            # ---------------- Phase 3: Wo + LN2 + FFN ----------------
            with (
                tc.tile_pool(name="lat2p", bufs=1) as lat2p,
                tc.tile_pool(name="nx2p", bufs=1) as nx2p,
                tc.tile_pool(name="sq2p", bufs=1) as sq2p,
                tc.tile_pool(name="ab2p", bufs=1) as ab2p,
                tc.tile_pool(name="small2p", bufs=2) as small2p,
                tc.tile_pool(name="lntmp2p", bufs=2) as lntmp2p,
                tc.tile_pool(name="w1sp", bufs=4) as w1sp,
                tc.tile_pool(name="w2sp", bufs=4) as w2sp,
                tc.tile_pool(name="hp_pool", bufs=4) as hp_pool,
                tc.tile_pool(name="outp", bufs=1) as outp,
            ):
                lat2T = lat2p.tile([P, FC, TQ], F32)
                with (
                    tc.tile_pool(name="ps_wo", bufs=2, space="PSUM") as ps_wo,
                    tc.tile_pool(name="ps_st2", bufs=1, space="PSUM") as ps_st2,
                ):
                    # Wo projection + residual
                    for mc in range(FC):
                        ps = ps_wo.tile([P, TQ], F32, tag="wo")
                        for kc in range(FC):
                            nc.tensor.matmul(ps[:], wo_sb[:, kc, ts(mc, P)],
                                             ctxT[:, kc, :],
                                             start=(kc == 0), stop=(kc == FC - 1))
                        nc.vector.affine_then_add(lat2T[:, mc, :], ps[:],
                                                  resid1[:, mc, :], 1.0,
                                                  bo_sb[:, mc:mc + 1])
                    # LN2
                    nx2T = nx2p.tile([P, FC, TQ], BF16)
                    sq2 = sq2p.tile([P, FC, TQ], BF16, tag="sq2")
                    latbf2 = sq2p.tile([P, FC, TQ], BF16, tag="latbf2")
                    _emit_ln(nc, TQ, lat2T[:], nx2T, latbf2, sq2, lntmp2p,
                             small2p, ab2p, ps_st2, ones_col_bf, eps_tile)

                outT = outp.tile([P, FC, TQ], F32)
                with (
                    tc.tile_pool(name="ps_fo", bufs=1, space="PSUM") as ps_fo,
                    tc.tile_pool(name="ps_h", bufs=2, space="PSUM") as ps_h,
                ):
                    ps_out = ps_fo.tile([P, FC, TQ], F32)
                    for mh in range(FFC):
                        w1t = w1sp.tile([P, FC, P], BF16, tag="w1s")
                        nc.sync.dma_start(
                            w1t[:], w1_d.ap()[mh].rearrange("(c p) m -> p c m", p=P))
                        w2t = w2sp.tile([P, H], BF16, tag="w2s")
                        nc.sync.dma_start(w2t[:], w2_d.ap()[ts(mh, P)])
                        psh = ps_h.tile([P, TQ], F32, tag="h")
                        for kc in range(FC):
                            nc.tensor.matmul(psh[:], w1t[:, kc, :], nx2T[:, kc, :],
                                             start=(kc == 0), stop=(kc == FC - 1))
                        h_t = hp_pool.tile([P, TQ], BF16, tag="h_sb")
                        nc.scalar.activation(h_t[:], psh[:], AF.Gelu,
                                             bias=b1_sb[:, mh:mh + 1])
                        for mc in range(FC):
                            nc.tensor.matmul(ps_out[:, mc, :], w2t[:, ts(mc, P)],
                                             h_t[:],
                                             start=(mh == 0), stop=(mh == FFC - 1))
                    for mc in range(FC):
                        nc.vector.affine_then_add(outT[:, mc, :], ps_out[:, mc, :],
                                                  lat2T[:, mc, :], 1.0,
                                                  b2_sb[:, mc:mc + 1])
                nc.sync.dma_start(out_ap, outT[:])

    nc.compile()
    return nc


_NC_CACHE = {}


def _get_nc():
    if "nc" not in _NC_CACHE:
        _NC_CACHE["nc"] = build()
    return _NC_CACHE["nc"]


def _prep_inputs(latent, ln1_w, ln1_b, Wq, bq, Wk, bk, Wv, bv, Wo, bo,
                 ln2_w, ln2_b, W1, b1, W2, b2):
    f32 = np.float32
    bf16 = ml_dtypes.bfloat16
    lat = np.asarray(latent, f32)
    ln1_w = np.asarray(ln1_w, f32); ln1_b = np.asarray(ln1_b, f32)
    ln2_w = np.asarray(ln2_w, f32); ln2_b = np.asarray(ln2_b, f32)
    Wq = np.asarray(Wq, f32); Wk = np.asarray(Wk, f32); Wv = np.asarray(Wv, f32)
    Wo = np.asarray(Wo, f32); W1 = np.asarray(W1, f32); W2 = np.asarray(W2, f32)
    bq = np.asarray(bq, f32); bk = np.asarray(bk, f32); bv = np.asarray(bv, f32)
    bo = np.asarray(bo, f32); b1 = np.asarray(b1, f32); b2 = np.asarray(b2, f32)

    wq_eff = (ln1_w[:, None] * Wq).astype(bf16)
    wk_eff = (ln1_w[:, None] * Wk).astype(bf16)
    wv_eff = (ln1_w[:, None] * Wv).astype(bf16)
    wo_bf = Wo.astype(bf16)
    bq_eff = ln1_b @ Wq + bq
    bk_eff = ln1_b @ Wk + bk
    bv_eff = ln1_b @ Wv + bv
    bo_eff = bv_eff @ Wo + bo
    w1_eff = ln2_w[:, None] * W1
    b1_eff = ln2_b @ W1 + b1
    w1r = np.ascontiguousarray(
        w1_eff.reshape(H, FFC, P).transpose(1, 0, 2)).astype(bf16)
    w2_bf = W2.astype(bf16)

    def chunked(b):  # [H] -> [P, FC]
        return np.ascontiguousarray(b.reshape(-1, P).T)

    common = {
        "wq": wq_eff, "wk": wk_eff, "wv": wv_eff, "wo": wo_bf,
        "w1r": w1r, "w2": w2_bf,
        "bq": chunked(bq_eff), "bk": chunked(bk_eff), "bo": chunked(bo_eff),
        "b1": chunked(b1_eff), "b2": chunked(b2),
    }
    in_maps = []
    for c in range(NCORES):
        b = c // (NCORES // B)
        q = c % (NCORES // B)
        latT_c = np.ascontiguousarray(np.roll(lat[b].T, -q * TQ, axis=1))
        m = dict(common)
        m["latT"] = latT_c
        in_maps.append(m)
    return in_maps


def kernel(**inputs):
    nc = _get_nc()
    in_maps = _prep_inputs(**inputs)
    res = run_bass_kernel_spmd(nc, in_maps, core_ids=list(range(NCORES)))
    out = np.empty((B, S, H), np.float32)
    for c in range(NCORES):
        b = c // (NCORES // B)
        q = c % (NCORES // B)
        out[b, q * TQ:(q + 1) * TQ, :] = res.results[c]["outT"].T
    return out


# revision 15
# speedup vs baseline: 1.1404x; 1.1404x over previous
"""Trainium2 Bass kernel for a BasicTransformerBlock (B=2, S=2048, H=768, FF=3072, NH=12).

Sharding: core c handles batch b=c//4, sequence quarter q=c%4 (512 tokens).
Each core redundantly computes LN1 + K/V projections for its batch's full
2048 tokens (no collectives needed); Q/attention/Wo/FFN only for its own 512
tokens.  Activations are kept feature-major ([feature, token]) on chip;
attention scores are computed transposed ([tk, tq]) so softmax reduces over
the partition dim via M=1 ones-matmuls (rowsums) and a deferred normalize.

LN affine params and all biases are folded host-side:
  Wq_eff = diag(ln1_w) Wq, bq_eff = ln1_b@Wq + bq  (same k)
  v carries no bias on device;  bo_eff = (ln1_b@Wv + bv)@Wo + bo
  W1_eff = diag(ln2_w) W1, b1_eff = ln2_b@W1 + b1
"""

import numpy as np
import ml_dtypes

import concourse.bass as bass
import concourse.tile as tile
from concourse import bacc, mybir
from concourse.bass import ts, ds
from concourse.alu_op_type import AluOpType
from concourse.bass_utils import run_bass_kernel_spmd

F32 = mybir.dt.float32
F32R = mybir.dt.float32r
BF16 = mybir.dt.bfloat16
AF = mybir.ActivationFunctionType

H = 768
FF = 3072
NH = 12
DH = 64
B = 2
S = 2048
P = 128
NCORES = 8
TQ = 512          # own tokens per core
NTT = S // TQ     # 4 token tiles per batch
FC = H // P       # 6 feature chunks
FFC = FF // P     # 24 hidden chunks
TKC = S // P      # 16 key token chunks
HPAIRS = NH // 2  # 6 head pairs
EPS = 1e-6


def _emit_ln(nc, T, lat_ap, nx_tile, lat_bf_tile, sq_tile, tmp_pool, small_pool,
             ab_pool, ps_stats_pool, ones_col_bf, eps_tile):
    """Feature-major layernorm: lat_ap/[P,FC,T] f32 -> nx_tile [P,FC,T] bf16."""
    # bf16 copies for the PE stat reductions (ACT + DVE)
    nc.scalar.copy(lat_bf_tile[:], lat_ap)
    nc.vector.tensor_mul(sq_tile[:], lat_ap, lat_ap)
    # per-token sum and sqsum via ones-matmuls accumulated in PSUM
    ps_stat = ps_stats_pool.tile([33, T], F32, tag="stats")
    for c in range(FC):
        nc.tensor.matmul(ps_stat[0:1, :], ones_col_bf[:],
                         lat_bf_tile[:, c, :],
                         start=(c == 0), stop=(c == FC - 1))
    for c in range(FC):
        nc.tensor.matmul(ps_stat[32:33, :], ones_col_bf[:],
                         sq_tile[:, c, :],
                         start=(c == 0), stop=(c == FC - 1))
    # mu, var, rsig, beta  (tiny [1,T] ops)
    mu = small_pool.tile([1, T], F32, tag="mu")
    nc.vector.tensor_scalar_mul(mu[:], ps_stat[0:1, :], 1.0 / H)
    msq = small_pool.tile([1, T], F32, tag="msq")
    nc.vector.tensor_scalar_mul(msq[:], ps_stat[32:33, :], 1.0 / H)
    var = small_pool.tile([1, T], F32, tag="var")
    nc.vector.tensor_mul(var[:], mu[:], mu[:])
    nc.vector.tensor_sub(var[:], msq[:], var[:])
    sd = small_pool.tile([1, T], F32, tag="sd")
    nc.scalar.activation(sd[:], var[:], AF.Sqrt, bias=eps_tile[:])
    rsig = small_pool.tile([1, T], F32, tag="rsig")
    nc.vector.reciprocal(rsig[:], sd[:])
    beta = small_pool.tile([1, T], F32, tag="beta")
    nc.vector.scalar_tensor_tensor(beta[:], mu[:], -1.0, rsig[:],
                                   AluOpType.mult, AluOpType.mult)
    # broadcast alpha/beta along partitions (GPSIMD)
    ab = ab_pool.tile([P, T], F32, tag="ab")
    nc.gpsimd.partition_broadcast(ab[:], rsig[:])
    bb = ab_pool.tile([P, T], F32, tag="bb")
    nc.gpsimd.partition_broadcast(bb[:], beta[:])
    # apply: nx = lat * ab + bb   (per chunk, DVE)
    for c in range(FC):
        t = tmp_pool.tile([P, T], F32, tag="lntmp")
        nc.vector.tensor_mul(t[:], lat_ap[:, c, :], ab[:])
        nc.vector.tensor_add(nx_tile[:, c, :], t[:], bb[:])


def build():
    nc = bacc.Bacc("TRN2", target_bir_lowering=False, debug=False,
                   num_devices=NCORES)

    latT_d = nc.dram_tensor("latT", [H, S], F32, kind="ExternalInput")
    wq_d = nc.dram_tensor("wq", [H, H], BF16, kind="ExternalInput")
    wk_d = nc.dram_tensor("wk", [H, H], BF16, kind="ExternalInput")
    wv_d = nc.dram_tensor("wv", [H, H], BF16, kind="ExternalInput")
    wo_d = nc.dram_tensor("wo", [H, H], BF16, kind="ExternalInput")
    w1_d = nc.dram_tensor("w1r", [FFC, H, P], BF16, kind="ExternalInput")
    w2_d = nc.dram_tensor("w2", [FF, H], BF16, kind="ExternalInput")
    bq_d = nc.dram_tensor("bq", [P, FC], F32, kind="ExternalInput")
    bk_d = nc.dram_tensor("bk", [P, FC], F32, kind="ExternalInput")
    bo_d = nc.dram_tensor("bo", [P, FC], F32, kind="ExternalInput")
    b1_d = nc.dram_tensor("b1", [P, FFC], F32, kind="ExternalInput")
    b2_d = nc.dram_tensor("b2", [P, FC], F32, kind="ExternalInput")
    out_d = nc.dram_tensor("outT", [H, TQ], F32, kind="ExternalOutput")

    latT_ap = latT_d.ap().rearrange("(c p) t -> p c t", p=P)
    out_ap = out_d.ap().rearrange("(c p) t -> p c t", p=P)

    with tile.TileContext(nc) as tc:
        with (
            tc.tile_pool(name="consts", bufs=1) as consts,
            tc.tile_pool(name="persist", bufs=1) as persist,
        ):
            # constants
            ones_col_bf = consts.tile([P, 1], BF16)
            nc.vector.memset(ones_col_bf[:], 1.0)
            eps_tile = consts.tile([1, 1], F32)
            nc.vector.memset(eps_tile[:], EPS)
            zero_col = consts.tile([P, 1], F32)
            nc.vector.memset(zero_col[:], 0.0)
            bq_sb = consts.tile([P, FC], F32)
            nc.sync.dma_start(bq_sb[:], bq_d.ap())
            bk_sb = consts.tile([P, FC], F32)
            nc.sync.dma_start(bk_sb[:], bk_d.ap())
            bo_sb = consts.tile([P, FC], F32)
            nc.sync.dma_start(bo_sb[:], bo_d.ap())
            b1_sb = consts.tile([P, FFC], F32)
            nc.sync.dma_start(b1_sb[:], b1_d.ap())
            b2_sb = consts.tile([P, FC], F32)
            nc.sync.dma_start(b2_sb[:], b2_d.ap())

            # persistent activations
            kT = persist.tile([P, FC, S], BF16)       # K^T, full batch
            v_sb = persist.tile([P, TKC, NH, DH + 1], BF16)  # V (+ones col) token-major
            nc.vector.memset(v_sb[:, :, :, DH:DH + 1], 1.0)
            qT = persist.tile([P, FC, TQ], BF16)      # Q^T, own tokens
            ctxT = persist.tile([P, FC, TQ], BF16)    # attention output^T
            resid1 = persist.tile([P, FC, TQ], F32)   # latT own tokens (f32)

            # projection weights (scalar-ring DMA so latT loads aren't queued
            # behind them on the sync HWDGE FIFO)
            wq_sb = persist.tile([P, FC, H], BF16)
            nc.scalar.dma_start(wq_sb[:], wq_d.ap().rearrange("(c p) m -> p c m", p=P))
            wk_sb = persist.tile([P, FC, H], BF16)
            nc.scalar.dma_start(wk_sb[:], wk_d.ap().rearrange("(c p) m -> p c m", p=P))
            wv_sb = persist.tile([P, FC, H], BF16)
            nc.scalar.dma_start(wv_sb[:], wv_d.ap().rearrange("(c p) m -> p c m", p=P))
            wo_sb = persist.tile([P, FC, H], BF16)
            nc.scalar.dma_start(wo_sb[:], wo_d.ap().rearrange("(c p) m -> p c m", p=P))

            # ---------------- Phase 1: LN1 + K/V/Q projections ----------------
            with (
                tc.tile_pool(name="latp", bufs=2) as latp,
                tc.tile_pool(name="sqp", bufs=1) as sqp,
                tc.tile_pool(name="nxp", bufs=2) as nxp,
                tc.tile_pool(name="abp", bufs=2) as abp,
                tc.tile_pool(name="smallp", bufs=2) as smallp,
                tc.tile_pool(name="lntmpp", bufs=2) as lntmpp,
                tc.tile_pool(name="ps_stats", bufs=2, space="PSUM") as ps_stats,
                tc.tile_pool(name="ps_kq", bufs=2, space="PSUM") as ps_kq,
                tc.tile_pool(name="ps_v", bufs=2, space="PSUM") as ps_v,
            ):
                for tt in range(NTT):
                    if tt == 0:
                        lat_t = resid1
                        nc.sync.dma_start(lat_t[:], latT_ap[:, :, ts(tt, TQ)])
                    else:
                        lat_t = latp.tile([P, FC, TQ], F32, tag="lat")
                        nc.sync.dma_start(lat_t[:], latT_ap[:, :, ts(tt, TQ)])
                    sq_t = sqp.tile([P, FC, TQ], BF16, tag="sq")
                    latbf_t = sqp.tile([P, FC, TQ], BF16, tag="latbf")
                    nx_t = nxp.tile([P, FC, TQ], BF16, tag="nx")
                    _emit_ln(nc, TQ, lat_t[:], nx_t, latbf_t, sq_t, lntmpp,
                             smallp, abp, ps_stats, ones_col_bf, eps_tile)

                    # K projection (feature-major out)
                    for mc in range(FC):
                        ps = ps_kq.tile([P, TQ], F32, tag="kq")
                        for kc in range(FC):
                            nc.tensor.matmul(ps[:], wk_sb[:, kc, ts(mc, P)],
                                             nx_t[:, kc, :],
                                             start=(kc == 0), stop=(kc == FC - 1))
                        nc.scalar.activation(kT[:, mc, ts(tt, TQ)], ps[:],
                                             AF.Identity, bias=bk_sb[:, mc:mc + 1])
                    # V projection (token-major out)
                    for tcl in range(TQ // P):
                        tcg = tt * (TQ // P) + tcl
                        for half in range(2):
                            ps = ps_v.tile([P, 384], F32, tag="v")
                            for kc in range(FC):
                                nc.tensor.matmul(ps[:], nx_t[:, kc, ts(tcl, P)],
                                                 wv_sb[:, kc, ds(half * 384, 384)],
                                                 start=(kc == 0), stop=(kc == FC - 1))
                            nc.vector.tensor_copy(
                                v_sb[:, tcg, ds(half * 6, 6), 0:DH],
                                ps[:].rearrange("p (h d) -> p h d", d=DH))
                    # Q projection (own tokens only; tt==0 holds own tokens)
                    if tt == 0:
                        for mc in range(FC):
                            ps = ps_kq.tile([P, TQ], F32, tag="kq")
                            for kc in range(FC):
                                nc.tensor.matmul(ps[:], wq_sb[:, kc, ts(mc, P)],
                                                 nx_t[:, kc, :],
                                                 start=(kc == 0), stop=(kc == FC - 1))
                            nc.scalar.activation(qT[:, mc, :], ps[:],
                                                 AF.Identity, bias=bq_sb[:, mc:mc + 1])

            # ---------------- Phase 2: attention ----------------
            # Two head-pairs interleaved; scores for a pair land in one
            # [P,2,TQ] PSUM tile (heads A/B row-tiled, concurrent), one Exp
            # evicts both; ctx matmuls use M=65 (ones column in v_sb) so
            # row 64 accumulates the softmax denominator for free.
            with (
                tc.tile_pool(name="attnp", bufs=8) as attnp,
                tc.tile_pool(name="rssb", bufs=4) as rssb,
                tc.tile_pool(name="rbp", bufs=4) as rbp,
                tc.tile_pool(name="stgp", bufs=2) as stgp,
                tc.tile_pool(name="ps_sc", bufs=2, space="PSUM") as ps_sc,
                tc.tile_pool(name="ps_ctx", bufs=1, space="PSUM") as ps_ctx,
            ):
                for hpg in range(HPAIRS // 2):
                    hps = (2 * hpg, 2 * hpg + 1)
                    ctx_tiles = {}
                    for hp in hps:
                        ctxA_ps = ps_ctx.tile([DH + 1, TQ], F32, tag=f"ctxA{hp % 2}")
                        ctxB_ps = ps_ctx.tile([DH + 1, TQ], F32, tag=f"ctxB{hp % 2}")
                        ctx_tiles[hp] = (ctxA_ps, ctxB_ps)
                    for j in range(TKC):
                        for hp in hps:
                            hA, hB = 2 * hp, 2 * hp + 1
                            sc = ps_sc.tile([P, 2, TQ], F32, tag="sc")
                            nc.tensor.matmul(sc[:, 0, :], kT[0:DH, hp, ts(j, P)],
                                             qT[0:DH, hp, :], start=True, stop=True)
                            nc.tensor.matmul(sc[:, 1, :], kT[DH:P, hp, ts(j, P)],
                                             qT[DH:P, hp, :], start=True, stop=True)
                            a2 = attnp.tile([P, 2, TQ], BF16, tag="attn")
                            nc.scalar.activation(a2[:], sc[:], AF.Exp, scale=0.125,
                                                 bias=zero_col[:])
                            ctxA_ps, ctxB_ps = ctx_tiles[hp]
                            nc.tensor.matmul(ctxA_ps[:], v_sb[:, j, hA, :],
                                             a2[:, 0, :],
                                             start=(j == 0), stop=(j == TKC - 1))
                            nc.tensor.matmul(ctxB_ps[:], v_sb[:, j, hB, :],
                                             a2[:, 1, :],
                                             start=(j == 0), stop=(j == TKC - 1))
                    for hp in hps:
                        ctxA_ps, ctxB_ps = ctx_tiles[hp]
                        # softmax denominators sit in row 64 of each ctx tile
                        rcA = rssb.tile([DH + 1, TQ], F32, tag="rcA")
                        nc.vector.reciprocal(rcA[DH:DH + 1, :],
                                             ctxA_ps[DH:DH + 1, :])
                        rcB = rssb.tile([DH + 1, TQ], F32, tag="rcB")
                        nc.vector.reciprocal(rcB[DH:DH + 1, :],
                                             ctxB_ps[DH:DH + 1, :])
                        rbA = rbp.tile([DH, TQ], F32, tag="rbA")
                        nc.sync.dma_start(
                            rbA[:],
                            rcA[DH:DH + 1, :].unsqueeze(1).broadcast_to((1, DH, TQ)))
                        rbB = rbp.tile([DH, TQ], F32, tag="rbB")
                        nc.sync.dma_start(
                            rbB[:],
                            rcB[DH:DH + 1, :].unsqueeze(1).broadcast_to((1, DH, TQ)))
                        nc.vector.tensor_mul(ctxT[0:DH, hp, :], ctxA_ps[0:DH, :],
                                             rbA[:])
                        stgB = stgp.tile([DH, TQ], BF16, tag="stgB")
                        nc.vector.tensor_mul(stgB[:], ctxB_ps[0:DH, :], rbB[:])
                        nc.sync.dma_start(ctxT[DH:P, hp, :], stgB[:])

            # ---------------- Phase 3: Wo + LN2 + FFN ----------------
            with (
                tc.tile_pool(name="lat2p", bufs=1) as lat2p,
                tc.tile_pool(name="nx2p", bufs=1) as nx2p,
                tc.tile_pool(name="sq2p", bufs=1) as sq2p,
                tc.tile_pool(name="ab2p", bufs=1) as ab2p,
                tc.tile_pool(name="small2p", bufs=2) as small2p,
                tc.tile_pool(name="lntmp2p", bufs=2) as lntmp2p,
                tc.tile_pool(name="w1sp", bufs=4) as w1sp,
                tc.tile_pool(name="w2sp", bufs=4) as w2sp,
                tc.tile_pool(name="hp_pool", bufs=4) as hp_pool,
                tc.tile_pool(name="outp", bufs=1) as outp,
            ):
                lat2T = lat2p.tile([P, FC, TQ], F32)
                with (
                    tc.tile_pool(name="ps_wo", bufs=2, space="PSUM") as ps_wo,
                    tc.tile_pool(name="ps_st2", bufs=1, space="PSUM") as ps_st2,
                ):
                    # Wo projection + residual
                    for mc in range(FC):
                        ps = ps_wo.tile([P, TQ], F32, tag="wo")
                        for kc in range(FC):
                            nc.tensor.matmul(ps[:], wo_sb[:, kc, ts(mc, P)],
                                             ctxT[:, kc, :],
                                             start=(kc == 0), stop=(kc == FC - 1))
                        nc.vector.affine_then_add(lat2T[:, mc, :], ps[:],
                                                  resid1[:, mc, :], 1.0,
                                                  bo_sb[:, mc:mc + 1])
                    # LN2
                    nx2T = nx2p.tile([P, FC, TQ], BF16)
                    sq2 = sq2p.tile([P, FC, TQ], BF16, tag="sq2")
                    latbf2 = sq2p.tile([P, FC, TQ], BF16, tag="latbf2")
                    _emit_ln(nc, TQ, lat2T[:], nx2T, latbf2, sq2, lntmp2p,
                             small2p, ab2p, ps_st2, ones_col_bf, eps_tile)

                outT = outp.tile([P, FC, TQ], F32)
                with (
                    tc.tile_pool(name="ps_fo", bufs=1, space="PSUM") as ps_fo,
                    tc.tile_pool(name="ps_h", bufs=2, space="PSUM") as ps_h,
                ):
                    ps_out = ps_fo.tile([P, FC, TQ], F32)
                    for mh in range(FFC):
                        w1t = w1sp.tile([P, FC, P], BF16, tag="w1s")
                        nc.sync.dma_start(
                            w1t[:], w1_d.ap()[mh].rearrange("(c p) m -> p c m", p=P))
                        w2t = w2sp.tile([P, H], BF16, tag="w2s")
                        nc.sync.dma_start(w2t[:], w2_d.ap()[ts(mh, P)])
                        psh = ps_h.tile([P, TQ], F32, tag="h")
                        for kc in range(FC):
                            nc.tensor.matmul(psh[:], w1t[:, kc, :], nx2T[:, kc, :],
                                             start=(kc == 0), stop=(kc == FC - 1))
                        h_t = hp_pool.tile([P, TQ], BF16, tag="h_sb")
                        nc.scalar.activation(h_t[:], psh[:], AF.Gelu,
                                             bias=b1_sb[:, mh:mh + 1])
                        for mc in range(FC):
                            nc.tensor.matmul(ps_out[:, mc, :], w2t[:, ts(mc, P)],
                                             h_t[:],
                                             start=(mh == 0), stop=(mh == FFC - 1))
                    for mc in range(FC):
                        nc.vector.affine_then_add(outT[:, mc, :], ps_out[:, mc, :],
                                                  lat2T[:, mc, :], 1.0,
                                                  b2_sb[:, mc:mc + 1])
                nc.sync.dma_start(out_ap, outT[:])

    nc.compile()
    return nc


_NC_CACHE = {}


def _get_nc():
    if "nc" not in _NC_CACHE:
        _NC_CACHE["nc"] = build()
    return _NC_CACHE["nc"]


def _prep_inputs(latent, ln1_w, ln1_b, Wq, bq, Wk, bk, Wv, bv, Wo, bo,
                 ln2_w, ln2_b, W1, b1, W2, b2):
    f32 = np.float32
    bf16 = ml_dtypes.bfloat16
    lat = np.asarray(latent, f32)
    ln1_w = np.asarray(ln1_w, f32); ln1_b = np.asarray(ln1_b, f32)
    ln2_w = np.asarray(ln2_w, f32); ln2_b = np.asarray(ln2_b, f32)
    Wq = np.asarray(Wq, f32); Wk = np.asarray(Wk, f32); Wv = np.asarray(Wv, f32)
    Wo = np.asarray(Wo, f32); W1 = np.asarray(W1, f32); W2 = np.asarray(W2, f32)
    bq = np.asarray(bq, f32); bk = np.asarray(bk, f32); bv = np.asarray(bv, f32)
    bo = np.asarray(bo, f32); b1 = np.asarray(b1, f32); b2 = np.asarray(b2, f32)

    wq_eff = (ln1_w[:, None] * Wq).astype(bf16)
    wk_eff = (ln1_w[:, None] * Wk).astype(bf16)
    wv_eff = (ln1_w[:, None] * Wv).astype(bf16)
    wo_bf = Wo.astype(bf16)
    bq_eff = ln1_b @ Wq + bq
    bk_eff = ln1_b @ Wk + bk
    bv_eff = ln1_b @ Wv + bv
    bo_eff = bv_eff @ Wo + bo
    w1_eff = ln2_w[:, None] * W1
    b1_eff = ln2_b @ W1 + b1
    w1r = np.ascontiguousarray(
        w1_eff.reshape(H, FFC, P).transpose(1, 0, 2)).astype(bf16)
    w2_bf = W2.astype(bf16)

    def chunked(b):  # [H] -> [P, FC]
        return np.ascontiguousarray(b.reshape(-1, P).T)

    common = {
        "wq": wq_eff, "wk": wk_eff, "wv": wv_eff, "wo": wo_bf,
        "w1r": w1r, "w2": w2_bf,
        "bq": chunked(bq_eff), "bk": chunked(bk_eff), "bo": chunked(bo_eff),
        "b1": chunked(b1_eff), "b2": chunked(b2),
    }
    in_maps = []
    for c in range(NCORES):
        b = c // (NCORES // B)
        q = c % (NCORES // B)
        latT_c = np.ascontiguousarray(np.roll(lat[b].T, -q * TQ, axis=1))
        m = dict(common)
        m["latT"] = latT_c
        in_maps.append(m)
    return in_maps


def kernel(**inputs):
    nc = _get_nc()
    in_maps = _prep_inputs(**inputs)
    res = run_bass_kernel_spmd(nc, in_maps, core_ids=list(range(NCORES)))
    out = np.empty((B, S, H), np.float32)
    for c in range(NCORES):
        b = c // (NCORES // B)
        q = c % (NCORES // B)
        out[b, q * TQ:(q + 1) * TQ, :] = res.results[c]["outT"].T
    return out


# revision 16
# speedup vs baseline: 1.2478x; 1.0942x over previous
"""Trainium2 Bass kernel for a BasicTransformerBlock (B=2, S=2048, H=768, FF=3072, NH=12).

Sharding: core c handles batch b=c//4, sequence quarter q=c%4 (512 tokens).
Each core redundantly computes LN1 + K/V projections for its batch's full
2048 tokens (no collectives needed); Q/attention/Wo/FFN only for its own 512
tokens.  Activations are kept feature-major ([feature, token]) on chip;
attention scores are computed transposed ([tk, tq]) so softmax reduces over
the partition dim via M=1 ones-matmuls (rowsums) and a deferred normalize.

LN affine params and all biases are folded host-side:
  Wq_eff = diag(ln1_w) Wq, bq_eff = ln1_b@Wq + bq  (same k)
  v carries no bias on device;  bo_eff = (ln1_b@Wv + bv)@Wo + bo
  W1_eff = diag(ln2_w) W1, b1_eff = ln2_b@W1 + b1
"""

import numpy as np
import ml_dtypes

import concourse.bass as bass
import concourse.tile as tile
from concourse import bacc, mybir
from concourse.bass import ts, ds
from concourse.alu_op_type import AluOpType
from concourse.bass_utils import run_bass_kernel_spmd

F32 = mybir.dt.float32
F32R = mybir.dt.float32r
BF16 = mybir.dt.bfloat16
AF = mybir.ActivationFunctionType

H = 768
FF = 3072
NH = 12
DH = 64
B = 2
S = 2048
P = 128
NCORES = 8
TQ = 512          # own tokens per core
NTT = S // TQ     # 4 token tiles per batch
FC = H // P       # 6 feature chunks
FFC = FF // P     # 24 hidden chunks
TKC = S // P      # 16 key token chunks
HPAIRS = NH // 2  # 6 head pairs
EPS = 1e-6


def _emit_ln(nc, T, lat_ap, nx_tile, lat_bf_tile, sq_tile, tmp_pool, small_pool,
             ab_pool, ps_stats_pool, ones_col_bf, eps_tile):
    """Feature-major layernorm: lat_ap/[P,FC,T] f32 -> nx_tile [P,FC,T] bf16."""
    # bf16 copies for the PE stat reductions (ACT + DVE)
    nc.scalar.copy(lat_bf_tile[:], lat_ap)
    nc.vector.tensor_mul(sq_tile[:], lat_ap, lat_ap)
    # per-token sum and sqsum via ones-matmuls accumulated in PSUM
    ps_stat = ps_stats_pool.tile([33, T], F32, tag="stats")
    for c in range(FC):
        nc.tensor.matmul(ps_stat[0:1, :], ones_col_bf[:],
                         lat_bf_tile[:, c, :],
                         start=(c == 0), stop=(c == FC - 1))
    for c in range(FC):
        nc.tensor.matmul(ps_stat[32:33, :], ones_col_bf[:],
                         sq_tile[:, c, :],
                         start=(c == 0), stop=(c == FC - 1))
    # mu, var, rsig, beta  (tiny [1,T] ops)
    mu = small_pool.tile([1, T], F32, tag="mu")
    nc.vector.tensor_scalar_mul(mu[:], ps_stat[0:1, :], 1.0 / H)
    msq = small_pool.tile([1, T], F32, tag="msq")
    nc.vector.tensor_scalar_mul(msq[:], ps_stat[32:33, :], 1.0 / H)
    var = small_pool.tile([1, T], F32, tag="var")
    nc.vector.tensor_mul(var[:], mu[:], mu[:])
    nc.vector.tensor_sub(var[:], msq[:], var[:])
    sd = small_pool.tile([1, T], F32, tag="sd")
    nc.scalar.activation(sd[:], var[:], AF.Sqrt, bias=eps_tile[:])
    rsig = small_pool.tile([1, T], F32, tag="rsig")
    nc.vector.reciprocal(rsig[:], sd[:])
    beta = small_pool.tile([1, T], F32, tag="beta")
    nc.vector.scalar_tensor_tensor(beta[:], mu[:], -1.0, rsig[:],
                                   AluOpType.mult, AluOpType.mult)
    # broadcast alpha/beta along partitions (GPSIMD)
    ab = ab_pool.tile([P, T], F32, tag="ab")
    nc.gpsimd.partition_broadcast(ab[:], rsig[:])
    bb = ab_pool.tile([P, T], F32, tag="bb")
    nc.gpsimd.partition_broadcast(bb[:], beta[:])
    # apply: nx = lat * ab + bb   (per chunk, DVE)
    for c in range(FC):
        t = tmp_pool.tile([P, T], F32, tag="lntmp")
        nc.vector.tensor_mul(t[:], lat_ap[:, c, :], ab[:])
        nc.vector.tensor_add(nx_tile[:, c, :], t[:], bb[:])


def build():
    nc = bacc.Bacc("TRN2", target_bir_lowering=False, debug=False,
                   num_devices=NCORES)

    latT_d = nc.dram_tensor("latT", [H, S], F32, kind="ExternalInput")
    wq_d = nc.dram_tensor("wq", [H, H], BF16, kind="ExternalInput")
    wk_d = nc.dram_tensor("wk", [H, H], BF16, kind="ExternalInput")
    wv_d = nc.dram_tensor("wv", [H, H], BF16, kind="ExternalInput")
    wo_d = nc.dram_tensor("wo", [H, H], BF16, kind="ExternalInput")
    w1_d = nc.dram_tensor("w1r", [FFC, H, P], BF16, kind="ExternalInput")
    w2_d = nc.dram_tensor("w2", [FF, H], BF16, kind="ExternalInput")
    bq_d = nc.dram_tensor("bq", [P, FC], F32, kind="ExternalInput")
    bk_d = nc.dram_tensor("bk", [P, FC], F32, kind="ExternalInput")
    bo_d = nc.dram_tensor("bo", [P, FC], F32, kind="ExternalInput")
    b1_d = nc.dram_tensor("b1", [P, FFC], F32, kind="ExternalInput")
    b2_d = nc.dram_tensor("b2", [P, FC], F32, kind="ExternalInput")
    out_d = nc.dram_tensor("outT", [H, TQ], F32, kind="ExternalOutput")

    latT_ap = latT_d.ap().rearrange("(c p) t -> p c t", p=P)
    out_ap = out_d.ap().rearrange("(c p) t -> p c t", p=P)

    with tile.TileContext(nc) as tc:
        with (
            tc.tile_pool(name="consts", bufs=1) as consts,
            tc.tile_pool(name="persist", bufs=1) as persist,
        ):
            # constants
            ones_col_bf = consts.tile([P, 1], BF16)
            nc.vector.memset(ones_col_bf[:], 1.0)
            eps_tile = consts.tile([1, 1], F32)
            nc.vector.memset(eps_tile[:], EPS)
            zero_col = consts.tile([P, 1], F32)
            nc.vector.memset(zero_col[:], 0.0)
            bq_sb = consts.tile([P, FC], F32)
            nc.sync.dma_start(bq_sb[:], bq_d.ap())
            bk_sb = consts.tile([P, FC], F32)
            nc.sync.dma_start(bk_sb[:], bk_d.ap())
            bo_sb = consts.tile([P, FC], F32)
            nc.sync.dma_start(bo_sb[:], bo_d.ap())
            b1_sb = consts.tile([P, FFC], F32)
            nc.sync.dma_start(b1_sb[:], b1_d.ap())
            b2_sb = consts.tile([P, FC], F32)
            nc.sync.dma_start(b2_sb[:], b2_d.ap())

            # persistent activations
            kT = persist.tile([P, FC, S], BF16)       # K^T, full batch
            v_sb = persist.tile([P, TKC, NH, DH + 1], BF16)  # V (+ones col) token-major
            nc.vector.memset(v_sb[:, :, :, DH:DH + 1], 1.0)
            qT = persist.tile([P, FC, TQ], BF16)      # Q^T, own tokens
            ctxT = persist.tile([P, FC, TQ], BF16)    # attention output^T
            resid1 = persist.tile([P, FC, TQ], F32)   # latT own tokens (f32)

            # projection weights (scalar-ring DMA so latT loads aren't queued
            # behind them on the sync HWDGE FIFO)
            wq_sb = persist.tile([P, FC, H], BF16)
            nc.scalar.dma_start(wq_sb[:], wq_d.ap().rearrange("(c p) m -> p c m", p=P))
            wk_sb = persist.tile([P, FC, H], BF16)
            nc.scalar.dma_start(wk_sb[:], wk_d.ap().rearrange("(c p) m -> p c m", p=P))
            wv_sb = persist.tile([P, FC, H], BF16)
            nc.scalar.dma_start(wv_sb[:], wv_d.ap().rearrange("(c p) m -> p c m", p=P))
            wo_sb = persist.tile([P, FC, H], BF16)
            nc.scalar.dma_start(wo_sb[:], wo_d.ap().rearrange("(c p) m -> p c m", p=P))

            # ---------------- Phase 1: LN1 + K/V/Q projections ----------------
            with (
                tc.tile_pool(name="latp", bufs=2) as latp,
                tc.tile_pool(name="sqp", bufs=1) as sqp,
                tc.tile_pool(name="nxp", bufs=2) as nxp,
                tc.tile_pool(name="abp", bufs=2) as abp,
                tc.tile_pool(name="smallp", bufs=2) as smallp,
                tc.tile_pool(name="lntmpp", bufs=2) as lntmpp,
                tc.tile_pool(name="ps_stats", bufs=2, space="PSUM") as ps_stats,
                tc.tile_pool(name="ps_kq", bufs=2, space="PSUM") as ps_kq,
                tc.tile_pool(name="ps_v", bufs=2, space="PSUM") as ps_v,
            ):
                for tt in range(NTT):
                    if tt == 0:
                        lat_t = resid1
                        nc.sync.dma_start(lat_t[:], latT_ap[:, :, ts(tt, TQ)])
                    else:
                        lat_t = latp.tile([P, FC, TQ], F32, tag="lat")
                        nc.sync.dma_start(lat_t[:], latT_ap[:, :, ts(tt, TQ)])
                    sq_t = sqp.tile([P, FC, TQ], BF16, tag="sq")
                    latbf_t = sqp.tile([P, FC, TQ], BF16, tag="latbf")
                    nx_t = nxp.tile([P, FC, TQ], BF16, tag="nx")
                    _emit_ln(nc, TQ, lat_t[:], nx_t, latbf_t, sq_t, lntmpp,
                             smallp, abp, ps_stats, ones_col_bf, eps_tile)

                    # K projection (feature-major out)
                    for mc in range(FC):
                        ps = ps_kq.tile([P, TQ], F32, tag="kq")
                        for kc in range(FC):
                            nc.tensor.matmul(ps[:], wk_sb[:, kc, ts(mc, P)],
                                             nx_t[:, kc, :],
                                             start=(kc == 0), stop=(kc == FC - 1))
                        nc.scalar.activation(kT[:, mc, ts(tt, TQ)], ps[:],
                                             AF.Identity, bias=bk_sb[:, mc:mc + 1])
                    # V projection (token-major out)
                    for tcl in range(TQ // P):
                        tcg = tt * (TQ // P) + tcl
                        for half in range(2):
                            ps = ps_v.tile([P, 384], F32, tag="v")
                            for kc in range(FC):
                                nc.tensor.matmul(ps[:], nx_t[:, kc, ts(tcl, P)],
                                                 wv_sb[:, kc, ds(half * 384, 384)],
                                                 start=(kc == 0), stop=(kc == FC - 1))
                            nc.vector.tensor_copy(
                                v_sb[:, tcg, ds(half * 6, 6), 0:DH],
                                ps[:].rearrange("p (h d) -> p h d", d=DH))
                    # Q projection (own tokens only; tt==0 holds own tokens)
                    if tt == 0:
                        for mc in range(FC):
                            ps = ps_kq.tile([P, TQ], F32, tag="kq")
                            for kc in range(FC):
                                nc.tensor.matmul(ps[:], wq_sb[:, kc, ts(mc, P)],
                                                 nx_t[:, kc, :],
                                                 start=(kc == 0), stop=(kc == FC - 1))
                            nc.scalar.activation(qT[:, mc, :], ps[:],
                                                 AF.Identity, bias=bq_sb[:, mc:mc + 1])

            # ---------------- Phase 2: attention ----------------
            # Two head-pairs interleaved; scores for a pair land in one
            # [P,2,TQ] PSUM tile (heads A/B row-tiled, concurrent), one Exp
            # evicts both; ctx matmuls use M=65 (ones column in v_sb) so
            # row 64 accumulates the softmax denominator for free.
            with (
                tc.tile_pool(name="attnp", bufs=8) as attnp,
                tc.tile_pool(name="rssb", bufs=4) as rssb,
                tc.tile_pool(name="rbp", bufs=4) as rbp,
                tc.tile_pool(name="stgp", bufs=2) as stgp,
                tc.tile_pool(name="ps_sc", bufs=2, space="PSUM") as ps_sc,
                tc.tile_pool(name="ps_ctx", bufs=1, space="PSUM") as ps_ctx,
            ):
                for hpg in range(HPAIRS // 2):
                    hps = (2 * hpg, 2 * hpg + 1)
                    ctx_tiles = {}
                    for hp in hps:
                        ctxA_ps = ps_ctx.tile([DH + 1, TQ], F32, tag=f"ctxA{hp % 2}")
                        ctxB_ps = ps_ctx.tile([DH + 1, TQ], F32, tag=f"ctxB{hp % 2}")
                        ctx_tiles[hp] = (ctxA_ps, ctxB_ps)
                    for j in range(TKC):
                        for hp in hps:
                            hA, hB = 2 * hp, 2 * hp + 1
                            sc = ps_sc.tile([P, 2, TQ], F32, tag="sc")
                            nc.tensor.matmul(sc[:, 0, :], kT[0:DH, hp, ts(j, P)],
                                             qT[0:DH, hp, :], start=True, stop=True)
                            nc.tensor.matmul(sc[:, 1, :], kT[DH:P, hp, ts(j, P)],
                                             qT[DH:P, hp, :], start=True, stop=True)
                            a2 = attnp.tile([P, 2, TQ], BF16, tag="attn")
                            nc.scalar.activation(a2[:], sc[:], AF.Exp, scale=0.125,
                                                 bias=zero_col[:])
                            ctxA_ps, ctxB_ps = ctx_tiles[hp]
                            nc.tensor.matmul(ctxA_ps[:], v_sb[:, j, hA, :],
                                             a2[:, 0, :],
                                             start=(j == 0), stop=(j == TKC - 1))
                            nc.tensor.matmul(ctxB_ps[:], v_sb[:, j, hB, :],
                                             a2[:, 1, :],
                                             start=(j == 0), stop=(j == TKC - 1))
                    for hp in hps:
                        ctxA_ps, ctxB_ps = ctx_tiles[hp]
                        # evict unnormalized ctx (+denominator row 64) to SBUF
                        # immediately so the PSUM banks free up for the next
                        # head-group; normalize asynchronously from SBUF.
                        cuA = stgp.tile([DH + 1, TQ], F32, tag="cuA")
                        nc.scalar.copy(cuA[:], ctxA_ps[:])
                        cuB = stgp.tile([DH + 1, TQ], F32, tag="cuB")
                        nc.scalar.copy(cuB[:], ctxB_ps[:])
                        rcA = rssb.tile([DH + 1, TQ], F32, tag="rcA")
                        nc.vector.reciprocal(rcA[DH:DH + 1, :], cuA[DH:DH + 1, :])
                        rcB = rssb.tile([DH + 1, TQ], F32, tag="rcB")
                        nc.vector.reciprocal(rcB[DH:DH + 1, :], cuB[DH:DH + 1, :])
                        rbA = rbp.tile([DH, TQ], F32, tag="rbA")
                        nc.sync.dma_start(
                            rbA[:],
                            rcA[DH:DH + 1, :].unsqueeze(1).broadcast_to((1, DH, TQ)))
                        rbB = rbp.tile([DH, TQ], F32, tag="rbB")
                        nc.sync.dma_start(
                            rbB[:],
                            rcB[DH:DH + 1, :].unsqueeze(1).broadcast_to((1, DH, TQ)))
                        nc.vector.tensor_mul(ctxT[0:DH, hp, :], cuA[0:DH, :],
                                             rbA[:])
                        stgB = stgp.tile([DH, TQ], BF16, tag="stgB")
                        nc.vector.tensor_mul(stgB[:], cuB[0:DH, :], rbB[:])
                        nc.sync.dma_start(ctxT[DH:P, hp, :], stgB[:])

            # ---------------- Phase 3: Wo + LN2 + FFN ----------------
            with (
                tc.tile_pool(name="lat2p", bufs=1) as lat2p,
                tc.tile_pool(name="nx2p", bufs=1) as nx2p,
                tc.tile_pool(name="sq2p", bufs=1) as sq2p,
                tc.tile_pool(name="ab2p", bufs=1) as ab2p,
                tc.tile_pool(name="small2p", bufs=2) as small2p,
                tc.tile_pool(name="lntmp2p", bufs=2) as lntmp2p,
                tc.tile_pool(name="w1sp", bufs=4) as w1sp,
                tc.tile_pool(name="w2sp", bufs=4) as w2sp,
                tc.tile_pool(name="hp_pool", bufs=4) as hp_pool,
                tc.tile_pool(name="outp", bufs=1) as outp,
            ):
                lat2T = lat2p.tile([P, FC, TQ], F32)
                with (
                    tc.tile_pool(name="ps_wo", bufs=2, space="PSUM") as ps_wo,
                    tc.tile_pool(name="ps_st2", bufs=1, space="PSUM") as ps_st2,
                ):
                    # Wo projection + residual, with LN2 stats interleaved
                    # per-chunk so the LN2 reduction overlaps Wo matmuls.
                    nx2T = nx2p.tile([P, FC, TQ], BF16)
                    sq2 = sq2p.tile([P, FC, TQ], BF16, tag="sq2")
                    latbf2 = sq2p.tile([P, FC, TQ], BF16, tag="latbf2")
                    ps_sum2 = ps_st2.tile([1, TQ], F32, tag="sum2")
                    ps_sq2 = ps_st2.tile([33, TQ], F32, tag="sqs2")
                    for mc in range(FC):
                        ps = ps_wo.tile([P, TQ], F32, tag="wo")
                        for kc in range(FC):
                            nc.tensor.matmul(ps[:], wo_sb[:, kc, ts(mc, P)],
                                             ctxT[:, kc, :],
                                             start=(kc == 0), stop=(kc == FC - 1))
                        nc.vector.affine_then_add(lat2T[:, mc, :], ps[:],
                                                  resid1[:, mc, :], 1.0,
                                                  bo_sb[:, mc:mc + 1])
                        nc.scalar.copy(latbf2[:, mc, :], lat2T[:, mc, :])
                        nc.vector.tensor_mul(sq2[:, mc, :], lat2T[:, mc, :],
                                             lat2T[:, mc, :])
                        nc.tensor.matmul(ps_sum2[0:1, :], ones_col_bf[:],
                                         latbf2[:, mc, :],
                                         start=(mc == 0), stop=(mc == FC - 1))
                        nc.tensor.matmul(ps_sq2[32:33, :], ones_col_bf[:],
                                         sq2[:, mc, :],
                                         start=(mc == 0), stop=(mc == FC - 1))
                    # LN2 tail: mu/var/rsig/beta + broadcast + apply
                    mu2 = small2p.tile([1, TQ], F32, tag="mu2")
                    nc.vector.tensor_scalar_mul(mu2[:], ps_sum2[0:1, :], 1.0 / H)
                    msq2 = small2p.tile([1, TQ], F32, tag="msq2")
                    nc.vector.tensor_scalar_mul(msq2[:], ps_sq2[32:33, :], 1.0 / H)
                    var2 = small2p.tile([1, TQ], F32, tag="var2")
                    nc.vector.tensor_mul(var2[:], mu2[:], mu2[:])
                    nc.vector.tensor_sub(var2[:], msq2[:], var2[:])
                    sd2 = small2p.tile([1, TQ], F32, tag="sd2")
                    nc.scalar.activation(sd2[:], var2[:], AF.Sqrt, bias=eps_tile[:])
                    rsig2 = small2p.tile([1, TQ], F32, tag="rsig2")
                    nc.vector.reciprocal(rsig2[:], sd2[:])
                    beta2 = small2p.tile([1, TQ], F32, tag="beta2")
                    nc.vector.scalar_tensor_tensor(beta2[:], mu2[:], -1.0, rsig2[:],
                                                   AluOpType.mult, AluOpType.mult)
                    ab2 = ab2p.tile([P, TQ], F32, tag="ab2")
                    nc.gpsimd.partition_broadcast(ab2[:], rsig2[:])
                    bb2 = ab2p.tile([P, TQ], F32, tag="bb2")
                    nc.gpsimd.partition_broadcast(bb2[:], beta2[:])
                    for c in range(FC):
                        t2 = lntmp2p.tile([P, TQ], F32, tag="lntmp2")
                        nc.vector.tensor_mul(t2[:], lat2T[:, c, :], ab2[:])
                        nc.vector.tensor_add(nx2T[:, c, :], t2[:], bb2[:])

                outT = outp.tile([P, FC, TQ], F32)
                with (
                    tc.tile_pool(name="ps_fo", bufs=1, space="PSUM") as ps_fo,
                    tc.tile_pool(name="ps_h", bufs=2, space="PSUM") as ps_h,
                ):
                    ps_out = ps_fo.tile([P, FC, TQ], F32)
                    for mh in range(FFC):
                        w1t = w1sp.tile([P, FC, P], BF16, tag="w1s")
                        nc.sync.dma_start(
                            w1t[:], w1_d.ap()[mh].rearrange("(c p) m -> p c m", p=P))
                        w2t = w2sp.tile([P, H], BF16, tag="w2s")
                        nc.sync.dma_start(w2t[:], w2_d.ap()[ts(mh, P)])
                        psh = ps_h.tile([P, TQ], F32, tag="h")
                        for kc in range(FC):
                            nc.tensor.matmul(psh[:], w1t[:, kc, :], nx2T[:, kc, :],
                                             start=(kc == 0), stop=(kc == FC - 1))
                        h_t = hp_pool.tile([P, TQ], BF16, tag="h_sb")
                        nc.scalar.activation(h_t[:], psh[:], AF.Gelu,
                                             bias=b1_sb[:, mh:mh + 1])
                        for mc in range(FC):
                            nc.tensor.matmul(ps_out[:, mc, :], w2t[:, ts(mc, P)],
                                             h_t[:],
                                             start=(mh == 0), stop=(mh == FFC - 1))
                    for mc in range(FC):
                        nc.vector.affine_then_add(outT[:, mc, :], ps_out[:, mc, :],
                                                  lat2T[:, mc, :], 1.0,
                                                  b2_sb[:, mc:mc + 1])
                nc.sync.dma_start(out_ap, outT[:])

    nc.compile()
    return nc


_NC_CACHE = {}


def _get_nc():
    if "nc" not in _NC_CACHE:
        _NC_CACHE["nc"] = build()
    return _NC_CACHE["nc"]


def _prep_inputs(latent, ln1_w, ln1_b, Wq, bq, Wk, bk, Wv, bv, Wo, bo,
                 ln2_w, ln2_b, W1, b1, W2, b2):
    f32 = np.float32
    bf16 = ml_dtypes.bfloat16
    lat = np.asarray(latent, f32)
    ln1_w = np.asarray(ln1_w, f32); ln1_b = np.asarray(ln1_b, f32)
    ln2_w = np.asarray(ln2_w, f32); ln2_b = np.asarray(ln2_b, f32)
    Wq = np.asarray(Wq, f32); Wk = np.asarray(Wk, f32); Wv = np.asarray(Wv, f32)
    Wo = np.asarray(Wo, f32); W1 = np.asarray(W1, f32); W2 = np.asarray(W2, f32)
    bq = np.asarray(bq, f32); bk = np.asarray(bk, f32); bv = np.asarray(bv, f32)
    bo = np.asarray(bo, f32); b1 = np.asarray(b1, f32); b2 = np.asarray(b2, f32)

    wq_eff = (ln1_w[:, None] * Wq).astype(bf16)
    wk_eff = (ln1_w[:, None] * Wk).astype(bf16)
    wv_eff = (ln1_w[:, None] * Wv).astype(bf16)
    wo_bf = Wo.astype(bf16)
    bq_eff = ln1_b @ Wq + bq
    bk_eff = ln1_b @ Wk + bk
    bv_eff = ln1_b @ Wv + bv
    bo_eff = bv_eff @ Wo + bo
    w1_eff = ln2_w[:, None] * W1
    b1_eff = ln2_b @ W1 + b1
    w1r = np.ascontiguousarray(
        w1_eff.reshape(H, FFC, P).transpose(1, 0, 2)).astype(bf16)
    w2_bf = W2.astype(bf16)

    def chunked(b):  # [H] -> [P, FC]
        return np.ascontiguousarray(b.reshape(-1, P).T)

    common = {
        "wq": wq_eff, "wk": wk_eff, "wv": wv_eff, "wo": wo_bf,
        "w1r": w1r, "w2": w2_bf,
        "bq": chunked(bq_eff), "bk": chunked(bk_eff), "bo": chunked(bo_eff),
        "b1": chunked(b1_eff), "b2": chunked(b2),
    }
    in_maps = []
    for c in range(NCORES):
        b = c // (NCORES // B)
        q = c % (NCORES // B)
        latT_c = np.ascontiguousarray(np.roll(lat[b].T, -q * TQ, axis=1))
        m = dict(common)
        m["latT"] = latT_c
        in_maps.append(m)
    return in_maps


def kernel(**inputs):
    nc = _get_nc()
    in_maps = _prep_inputs(**inputs)
    res = run_bass_kernel_spmd(nc, in_maps, core_ids=list(range(NCORES)))
    out = np.empty((B, S, H), np.float32)
    for c in range(NCORES):
        b = c // (NCORES // B)
        q = c % (NCORES // B)
        out[b, q * TQ:(q + 1) * TQ, :] = res.results[c]["outT"].T
    return out
